# revision 15
# baseline (speedup 1.0000x reference)
"""MoE layer (top-2 of 8 experts, d_model=1024, d_hidden=512) on 8 trn2 cores.

Expert-parallel with host-side dispatch/combine: the host computes gating in
float64 (exact routing), gathers each expert's assigned tokens into a padded
[D, CAP] buffer, and each core runs a single expert's 2-layer MLP in fp16
(fp32 PSUM accumulate). The host combines a token's two expert contributions
as p1*y1 + p2*y2 with two fancy-indexed gathers (experts distinct, so the
math matches the reference up to fp16 rounding in the MLP).

This computes only the top-2 selected expert-token pairs (1/4 of the dense
reference einsum). CAP is the exact max expert load (no 128-padding); both
matmuls stream tokens as the PSUM moving dim, so cost scales with real
tokens and the tail chunk is cheap.

Perf notes (from trace analysis of v1):
  - each dma_start costs ~0.6-1us serialized on its trigger engine; input
    triggers are split across Sync and Scalar (both HWDGE-capable) so the
    two first-needed tiles (w1 block 0, x chunk 0) are triggered in parallel
    and the PE starts ~6us earlier.
  - w1 arrives host-preblocked per 128-col block (contiguous 2KB lines) so
    the first matmul only waits on 256KB + the first small token chunk.
  - mm2 produces [d_block, tokens] (w2 stationary), so the last emitted
    chunk is small -> tiny tail drain; PSUM drains alternate vector/gpsimd.
"""

import os
import sys

import numpy as np

for _p in ("/opt/trn_rl_repo", "/root/.axon_site/_ro/trn_rl_repo"):
    if _p not in sys.path and os.path.isdir(_p):
        sys.path.append(_p)

P = 128
D_MODEL = 1024
C_HID = 512
N_EXP = 8
N_CORES = 8
T_FULL = 4 * 2048

KC = D_MODEL // P  # 8 contraction chunks over D (mm1)
CC = C_HID // P    # 4 contraction chunks over C (mm2)
DB = D_MODEL // P  # 8 output d_model blocks (mm2)

_CACHE = {}

# set by test harness to capture profiling info
TRACE = False
LAST_RESULT = None


def _install_ntff_hook_shim():
    """Register the axon NTFF profile hook if the image's antenv lacks it.

    bass_utils resolves the hook via `antenv.axon_hooks`; when that module is
    absent, tracing silently degrades. The hook implementation itself ships
    with the axon boot package, so wire it up through sys.modules.
    """
    try:
        from antenv.axon_hooks import get_axon_ntff_profile_hook  # noqa: F401
        return  # real module present
    except ImportError:
        pass
    try:
        import types

        if "/root/.axon_site" not in sys.path and os.path.isdir("/root/.axon_site"):
            sys.path.append("/root/.axon_site")
        from trn_agent_boot.trn_boot import _ntff_profile_via_ctypes

        so_path = "/opt/axon/libaxon_pjrt.so"
        if not os.path.exists(so_path):
            return
        hook = _ntff_profile_via_ctypes(so_path)
        mod = types.ModuleType("antenv.axon_hooks")
        mod.get_axon_ntff_profile_hook = lambda: hook
        mod.set_axon_ntff_profile_hook = lambda h: None
        import antenv

        antenv.axon_hooks = mod
        sys.modules["antenv.axon_hooks"] = mod
    except Exception:
        pass


def _split_excess_waits(nc, mybir, maxw=1):
    """This walrus build accepts at most one semaphore wait per instruction.

    Tile emits instructions (notably the kernel-tail drain) with several
    waits; split the extras into preceding single-wait NoOps on the same
    engine — program order makes the chain equivalent.
    """
    for f in nc.m.functions:
        for bb in f.blocks:
            out = []
            changed = False
            for ins in bb.instructions:
                si = ins.sync_info
                waits = list(si.on_wait) if (si is not None and si.on_wait) else []
                if len(waits) > maxw:
                    extra, keep = waits[:-maxw], waits[-maxw:]
                    for ci in range(0, len(extra), maxw):
                        out.append(mybir.InstNoOp(
                            name=f"{ins.name}_ws{ci}",
                            sync_info=mybir.SyncInfo(
                                on_wait=list(extra[ci:ci + maxw]), on_update=[]
                            ),
                            engine=ins.engine,
                            bass_nofuse=True,
                        ))
                    si.on_wait = keep
                    changed = True
                out.append(ins)
            if changed:
                bb.instructions = out
    return nc


def _make_chunks(cap):
    """Token chunks, each <=512 (PSUM moving limit). First chunk small so the
    PE starts as soon as ~512KB has landed; last chunk small so the final
    drain+DMA after the last matmul is tiny."""
    chunks = []
    first = min(256, cap)
    chunks.append(first)
    rem = cap - first
    while rem > 640:
        chunks.append(512)
        rem -= 512
    if rem > 128:
        chunks.append(rem - 128)
        rem = 128
    if rem:
        chunks.append(rem)
    out = []
    off = 0
    for n in chunks:
        out.append((off, n))
        off += n
    assert off == cap
    return out


def _build_nc(cap):
    import concourse.bass as bass
    import concourse.mybir as mybir
    import concourse.tile as tile
    from contextlib import ExitStack

    dt = mybir.dt
    f32 = dt.float32
    f16 = dt.float16
    OP = mybir.AluOpType
    ACT = mybir.ActivationFunctionType

    chunks = _make_chunks(cap)

    nc = bass.Bass("TRN2", debug=False)

    # All dram layouts are host-pre-blocked so every DMA moves long
    # per-partition-contiguous lines (128 lines per transfer instead of
    # 1024): DMA wall time is line-count dominated, so this is ~8x fewer
    # descriptors and a much faster first-tile landing.
    # xgp: chunk-major gathered tokens; chunk c occupies
    # [:, KC*o : KC*(o+n)] as [p][kc][t-in-chunk].
    xgp = nc.dram_tensor("xgp", [P, KC * cap], f16, kind="ExternalInput")
    # w1p: [cm, p, kc*128] — one contiguous 2KB line per partition per block
    w1p = nc.dram_tensor("w1p", [CC, P, KC * P], f16, kind="ExternalInput")
    # w2p: [p, cc*1024] — pre-blocked like w1
    w2p = nc.dram_tensor("w2p", [P, CC * D_MODEL], f16, kind="ExternalInput")
    # out: chunk-major [p][dblk][t-in-chunk] per chunk; host decodes.
    out = nc.dram_tensor("out", [P, DB * cap], f16, kind="ExternalOutput")

    with tile.TileContext(nc) as tc:
        with ExitStack() as ctx:
            cpool = ctx.enter_context(tc.tile_pool(name="cpool", bufs=1))
            opool = ctx.enter_context(tc.tile_pool(name="opool", bufs=4))
            psum_mm = ctx.enter_context(
                tc.tile_pool(name="psum_mm", bufs=6, space="PSUM"))

            xgc_sb = [
                cpool.tile([P, KC, n], f16, name=f"xg{ci}_sb")
                for ci, (o, n) in enumerate(chunks)
            ]
            w1_sb = cpool.tile([P, CC, KC * P], f16, name="w1_sb")
            w2_sb = cpool.tile([P, CC, D_MODEL], f16, name="w2_sb")
            ht_sb = cpool.tile([P, CC, cap], f16, name="ht_sb")

            # DMA triggers ordered by PE need, split across the two HWDGE
            # engines (scalar streams weights, sync streams tokens) so the
            # two first-needed tiles are triggered in parallel.
            nc.scalar.dma_start(w1_sb[:, 0, :], w1p[0])
            o0, n0 = chunks[0]
            nc.sync.dma_start(
                xgc_sb[0][:], xgp[:, KC * o0:KC * (o0 + n0)])
            for cm in range(1, CC):
                nc.scalar.dma_start(w1_sb[:, cm, :], w1p[cm])
            for ci, (o, n) in enumerate(chunks[1:], start=1):
                nc.sync.dma_start(
                    xgc_sb[ci][:], xgp[:, KC * o:KC * (o + n)])
            nc.scalar.dma_start(w2_sb[:], w2p[:])

            # PSUM drains alternate vector/scalar so neither engine sits on
            # the critical path (gpsimd cannot access PSUM).
            drain_state = [0]

            # mm1(c): hT[C, tokens-chunk] = relu(w1.T @ x), contract over D.
            # (The gate probability is applied by the host at combine time.)
            def emit_mm1(ci, o, n):
                for cm in range(CC):
                    ps_h = psum_mm.tile([P, 512], f32, name="ps_h", tag="ps")
                    for kc in range(KC):
                        nc.tensor.matmul(
                            ps_h[:, 0:n],
                            lhsT=w1_sb[:, cm, kc * P:(kc + 1) * P],
                            rhs=xgc_sb[ci][:, kc, :],
                            start=(kc == 0),
                            stop=(kc == KC - 1),
                        )
                    if drain_state[0] % 2 == 0:
                        nc.vector.tensor_scalar_max(
                            ht_sb[:, cm, o:o + n], ps_h[:, 0:n], 0.0)
                    else:
                        nc.scalar.activation(
                            ht_sb[:, cm, o:o + n], ps_h[:, 0:n], ACT.Relu)
                    drain_state[0] += 1

            # mm2(c): yT[D, tokens-chunk] = w2.T @ hT, contract over C.
            # Tokens are the moving dim, so per-chunk cost tracks real
            # tokens; the output DMA goes out in two dblk-halves (each a
            # contiguous 128-line transfer) so data starts leaving after
            # half the drains. The very last half triggers on scalar,
            # which is idle at the tail while sync finishes the prior one.
            def emit_mm2(ci, o, n, last=False):
                yt = opool.tile([P, DB, n], f16, name="yt", tag="yt")
                for dblk in range(DB):
                    ps_y = psum_mm.tile([P, 512], f32, name="ps_y", tag="ps")
                    for cc in range(CC):
                        nc.tensor.matmul(
                            ps_y[:, 0:n],
                            lhsT=w2_sb[:, cc, dblk * P:(dblk + 1) * P],
                            rhs=ht_sb[:, cc, o:o + n],
                            start=(cc == 0),
                            stop=(cc == CC - 1),
                        )
                    if drain_state[0] % 2 == 0:
                        nc.vector.tensor_copy(yt[:, dblk, :], ps_y[:, 0:n])
                    else:
                        nc.scalar.activation(
                            yt[:, dblk, :], ps_y[:, 0:n], ACT.Copy)
                    drain_state[0] += 1
                    if dblk == DB // 2 - 1 or dblk == DB - 1:
                        h0 = 0 if dblk == DB // 2 - 1 else DB // 2
                        eng = nc.scalar if (last and h0) else nc.sync
                        eng.dma_start(
                            out[:, DB * o + h0 * n:DB * o + (dblk + 1) * n],
                            yt[:, h0:dblk + 1, :])

            # Interleave with one chunk of lag: mm2(c) is emitted after
            # mm1(c+1), so mm1(c)'s drains are long done when mm2(c) hits
            # the PE, and output DMA spreads across the whole kernel
            # instead of piling up at the end.
            emit_mm1(0, *chunks[0])
            for ci in range(1, len(chunks)):
                emit_mm1(ci, *chunks[ci])
                emit_mm2(ci - 1, *chunks[ci - 1])
            emit_mm2(len(chunks) - 1, *chunks[-1], last=True)

    _split_excess_waits(nc, mybir)
    return nc


def _get_nc(cap):
    key = ("nc", cap)
    if key not in _CACHE:
        _CACHE[key] = _build_nc(cap)
    return _CACHE[key]


def kernel(**inputs) -> np.ndarray:
    global LAST_RESULT
    x = np.asarray(inputs["x"], dtype=np.float32)
    Wg = np.asarray(inputs["Wg"], dtype=np.float32)
    W1 = np.asarray(inputs["W1"], dtype=np.float32)
    W2 = np.asarray(inputs["W2"], dtype=np.float32)

    B, S, D = x.shape
    T = B * S
    xf = x.reshape(T, D)

    # ---- routing on host in float64 (logit gaps >> fp32 matmul noise, so
    # this reproduces the reference's fp32 top-2 decisions exactly)
    logits = xf.astype(np.float64) @ Wg.astype(np.float64)
    rows = np.arange(T)
    i1 = logits.argmax(1)
    l1 = logits[rows, i1]
    masked = logits.copy()
    masked[rows, i1] = -np.inf
    i2 = masked.argmax(1)
    l2 = masked[rows, i2]
    p2 = 1.0 / (1.0 + np.exp(l1 - l2))
    p1 = 1.0 - p2

    # ---- dispatch: per-expert token lists + slot of each token in its
    # expert's buffer (for the combine gather)
    slot1 = np.empty(T, np.int64)
    slot2 = np.empty(T, np.int64)
    idx_e = []
    gates_e = []
    for e in range(N_EXP):
        a = np.nonzero(i1 == e)[0]
        b = np.nonzero(i2 == e)[0]
        slot1[a] = np.arange(len(a))
        slot2[b] = len(a) + np.arange(len(b))
        idx_e.append(np.concatenate([a, b]))
        gates_e.append(np.concatenate([p1[a], p2[b]]).astype(np.float32))
    cap = max(max(len(ix) for ix in idx_e), 64)

    chunks = _make_chunks(cap)
    in_maps = []
    for e in range(N_EXP):
        n_e = len(idx_e[e])
        xg = np.zeros((D_MODEL, cap), np.float16)
        xg[:, :n_e] = xf[idx_e[e]].T.astype(np.float16)
        # chunk-major layout: each chunk contiguous per partition as
        # [p][kc][t-in-chunk]
        xgp = np.concatenate(
            [
                xg[:, o:o + n].reshape(KC, P, n)
                .transpose(1, 0, 2).reshape(P, KC * n)
                for o, n in chunks
            ],
            axis=1,
        )
        in_maps.append({
            "xgp": np.ascontiguousarray(xgp),
            "w1p": np.ascontiguousarray(
                W1[e].astype(np.float16)
                .reshape(KC, P, CC, P).transpose(2, 1, 0, 3)
                .reshape(CC, P, KC * P)),
            "w2p": np.ascontiguousarray(
                W2[e].astype(np.float16)
                .reshape(CC, P, D_MODEL).transpose(1, 0, 2)
                .reshape(P, CC * D_MODEL)),
        })

    from concourse.bass_utils import run_bass_kernel_spmd

    _install_ntff_hook_shim()
    nc = _get_nc(cap)
    res = run_bass_kernel_spmd(
        nc, in_maps, core_ids=list(range(N_CORES)), trace=TRACE
    )
    LAST_RESULT = res

    # ---- decode the chunk-major outputs back to [E, D, cap]
    y_all = np.empty((N_EXP, D_MODEL, cap), np.float16)
    for e in range(N_EXP):
        raw = res.results[e]["out"]  # [P, DB*cap] chunk-major
        for o, n in chunks:
            y_all[e, :, o:o + n] = (
                raw[:, DB * o:DB * (o + n)].reshape(P, DB, n)
                .transpose(1, 0, 2).reshape(D_MODEL, n))

    # ---- combine: token t = p1*y[e1] + p2*y[e2] via two fancy-indexed
    # gathers; gate probabilities applied here in fp32.
    c1 = y_all[i1, :, slot1].astype(np.float32)        # [T, D]
    c2 = y_all[i2, :, slot2].astype(np.float32)
    out = p1[:, None].astype(np.float32) * c1 + p2[:, None].astype(np.float32) * c2
    return out.astype(np.float32).reshape(B, S, D)


# revision 16
# speedup vs baseline: 1.1186x; 1.1186x over previous
"""MoE layer (top-2 of 8 experts, d_model=1024, d_hidden=512) on 8 trn2 cores.

Expert-parallel with host-side dispatch/combine: the host computes gating in
float64 (exact routing), gathers each expert's assigned tokens into a padded
[D, CAP] buffer, and each core runs a single expert's 2-layer MLP in fp16
(fp32 PSUM accumulate). The host combines a token's two expert contributions
as p1*y1 + p2*y2 with two fancy-indexed gathers (experts distinct, so the
math matches the reference up to fp16 rounding in the MLP).

This computes only the top-2 selected expert-token pairs (1/4 of the dense
reference einsum). CAP is the exact max expert load (no 128-padding); both
matmuls stream tokens as the PSUM moving dim, so cost scales with real
tokens and the tail chunk is cheap.

Perf notes (from trace analysis of v1):
  - each dma_start costs ~0.6-1us serialized on its trigger engine; input
    triggers are split across Sync and Scalar (both HWDGE-capable) so the
    two first-needed tiles (w1 block 0, x chunk 0) are triggered in parallel
    and the PE starts ~6us earlier.
  - w1 arrives host-preblocked per 128-col block (contiguous 2KB lines) so
    the first matmul only waits on 256KB + the first small token chunk.
  - mm2 produces [d_block, tokens] (w2 stationary), so the last emitted
    chunk is small -> tiny tail drain; PSUM drains alternate vector/gpsimd.
"""

import os
import sys

import numpy as np

for _p in ("/opt/trn_rl_repo", "/root/.axon_site/_ro/trn_rl_repo"):
    if _p not in sys.path and os.path.isdir(_p):
        sys.path.append(_p)

P = 128
D_MODEL = 1024
C_HID = 512
N_EXP = 8
N_CORES = 8
T_FULL = 4 * 2048

KC = D_MODEL // P  # 8 contraction chunks over D (mm1)
CC = C_HID // P    # 4 contraction chunks over C (mm2)
DB = D_MODEL // P  # 8 output d_model blocks (mm2)

_CACHE = {}

# set by test harness to capture profiling info
TRACE = False
LAST_RESULT = None


def _install_ntff_hook_shim():
    """Register the axon NTFF profile hook if the image's antenv lacks it.

    bass_utils resolves the hook via `antenv.axon_hooks`; when that module is
    absent, tracing silently degrades. The hook implementation itself ships
    with the axon boot package, so wire it up through sys.modules.
    """
    try:
        from antenv.axon_hooks import get_axon_ntff_profile_hook  # noqa: F401
        return  # real module present
    except ImportError:
        pass
    try:
        import types

        if "/root/.axon_site" not in sys.path and os.path.isdir("/root/.axon_site"):
            sys.path.append("/root/.axon_site")
        from trn_agent_boot.trn_boot import _ntff_profile_via_ctypes

        so_path = "/opt/axon/libaxon_pjrt.so"
        if not os.path.exists(so_path):
            return
        hook = _ntff_profile_via_ctypes(so_path)
        mod = types.ModuleType("antenv.axon_hooks")
        mod.get_axon_ntff_profile_hook = lambda: hook
        mod.set_axon_ntff_profile_hook = lambda h: None
        import antenv

        antenv.axon_hooks = mod
        sys.modules["antenv.axon_hooks"] = mod
    except Exception:
        pass


def _split_excess_waits(nc, mybir, maxw=1):
    """This walrus build accepts at most one semaphore wait per instruction.

    Tile emits instructions (notably the kernel-tail drain) with several
    waits; split the extras into preceding single-wait NoOps on the same
    engine — program order makes the chain equivalent.
    """
    for f in nc.m.functions:
        for bb in f.blocks:
            out = []
            changed = False
            for ins in bb.instructions:
                si = ins.sync_info
                waits = list(si.on_wait) if (si is not None and si.on_wait) else []
                if len(waits) > maxw:
                    extra, keep = waits[:-maxw], waits[-maxw:]
                    for ci in range(0, len(extra), maxw):
                        out.append(mybir.InstNoOp(
                            name=f"{ins.name}_ws{ci}",
                            sync_info=mybir.SyncInfo(
                                on_wait=list(extra[ci:ci + maxw]), on_update=[]
                            ),
                            engine=ins.engine,
                            bass_nofuse=True,
                        ))
                    si.on_wait = keep
                    changed = True
                out.append(ins)
            if changed:
                bb.instructions = out
    return nc


def _make_chunks(cap):
    """Token chunks, each <=512 (PSUM moving limit). First chunk small so the
    PE starts as soon as ~512KB has landed; last chunk small so the final
    drain+DMA after the last matmul is tiny."""
    chunks = []
    first = min(256, cap)
    chunks.append(first)
    rem = cap - first
    while rem > 640:
        chunks.append(512)
        rem -= 512
    if rem > 128:
        chunks.append(rem - 128)
        rem = 128
    if rem:
        chunks.append(rem)
    out = []
    off = 0
    for n in chunks:
        out.append((off, n))
        off += n
    assert off == cap
    return out


def _build_nc(cap):
    import concourse.bass as bass
    import concourse.mybir as mybir
    import concourse.tile as tile
    from contextlib import ExitStack

    dt = mybir.dt
    f32 = dt.float32
    f16 = dt.float16
    OP = mybir.AluOpType
    ACT = mybir.ActivationFunctionType

    chunks = _make_chunks(cap)

    nc = bass.Bass("TRN2", debug=False)

    # All dram layouts are host-pre-blocked so every DMA moves long
    # per-partition-contiguous lines (128 lines per transfer instead of
    # 1024): DMA wall time is line-count dominated, so this is ~8x fewer
    # descriptors and a much faster first-tile landing.
    # xgp: chunk-major gathered tokens; chunk c occupies
    # [:, KC*o : KC*(o+n)] as [p][kc][t-in-chunk].
    xgp = nc.dram_tensor("xgp", [P, KC * cap], f16, kind="ExternalInput")
    # w1p: [cm, p, kc*128] — one contiguous 2KB line per partition per block
    w1p = nc.dram_tensor("w1p", [CC, P, KC * P], f16, kind="ExternalInput")
    # w2p: [p, cc*1024] — pre-blocked like w1
    w2p = nc.dram_tensor("w2p", [P, CC * D_MODEL], f16, kind="ExternalInput")
    # out: chunk-major [p][dblk][t-in-chunk] per chunk; host decodes.
    out = nc.dram_tensor("out", [P, DB * cap], f16, kind="ExternalOutput")

    with tile.TileContext(nc) as tc:
        with ExitStack() as ctx:
            cpool = ctx.enter_context(tc.tile_pool(name="cpool", bufs=1))
            opool = ctx.enter_context(tc.tile_pool(name="opool", bufs=4))
            psum_mm = ctx.enter_context(
                tc.tile_pool(name="psum_mm", bufs=6, space="PSUM"))

            xgc_sb = [
                cpool.tile([P, KC, n], f16, name=f"xg{ci}_sb")
                for ci, (o, n) in enumerate(chunks)
            ]
            w1_sb = cpool.tile([P, CC, KC * P], f16, name="w1_sb")
            w2_sb = cpool.tile([P, CC, D_MODEL], f16, name="w2_sb")
            ht_sb = cpool.tile([P, CC, cap], f16, name="ht_sb")

            # DMA triggers in exact PE-need order on ONE queue (sync), so
            # ring FIFO order matches consumption order; only the very
            # first weight block rides the scalar queue in parallel with
            # the first token chunk.
            nc.scalar.dma_start(w1_sb[:, 0, :], w1p[0])
            o0, n0 = chunks[0]
            nc.sync.dma_start(
                xgc_sb[0][:], xgp[:, KC * o0:KC * (o0 + n0)])
            for cm in range(1, CC):
                nc.sync.dma_start(w1_sb[:, cm, :], w1p[cm])
            o1, n1 = chunks[1]
            nc.sync.dma_start(
                xgc_sb[1][:], xgp[:, KC * o1:KC * (o1 + n1)])
            nc.sync.dma_start(w2_sb[:], w2p[:])
            for ci, (o, n) in enumerate(chunks[2:], start=2):
                nc.sync.dma_start(
                    xgc_sb[ci][:], xgp[:, KC * o:KC * (o + n)])

            # PSUM drains alternate vector/scalar so neither engine sits on
            # the critical path (gpsimd cannot access PSUM).
            drain_state = [0]

            # mm1(c): hT[C, tokens-chunk] = relu(w1.T @ x), contract over D.
            # (The gate probability is applied by the host at combine time.)
            def emit_mm1(ci, o, n):
                for cm in range(CC):
                    ps_h = psum_mm.tile([P, 512], f32, name="ps_h", tag="ps")
                    for kc in range(KC):
                        nc.tensor.matmul(
                            ps_h[:, 0:n],
                            lhsT=w1_sb[:, cm, kc * P:(kc + 1) * P],
                            rhs=xgc_sb[ci][:, kc, :],
                            start=(kc == 0),
                            stop=(kc == KC - 1),
                        )
                    if drain_state[0] % 2 == 0:
                        nc.vector.tensor_scalar_max(
                            ht_sb[:, cm, o:o + n], ps_h[:, 0:n], 0.0)
                    else:
                        nc.scalar.activation(
                            ht_sb[:, cm, o:o + n], ps_h[:, 0:n], ACT.Relu)
                    drain_state[0] += 1

            # mm2(c): yT[D, tokens-chunk] = w2.T @ hT, contract over C.
            # Tokens are the moving dim, so per-chunk cost tracks real
            # tokens; the output DMA goes out in two dblk-halves (each a
            # contiguous 128-line transfer) so data starts leaving after
            # half the drains. The very last half triggers on scalar,
            # which is idle at the tail while sync finishes the prior one.
            def emit_mm2(ci, o, n, last=False):
                yt = opool.tile([P, DB, n], f16, name="yt", tag="yt")
                for dblk in range(DB):
                    ps_y = psum_mm.tile([P, 512], f32, name="ps_y", tag="ps")
                    for cc in range(CC):
                        nc.tensor.matmul(
                            ps_y[:, 0:n],
                            lhsT=w2_sb[:, cc, dblk * P:(dblk + 1) * P],
                            rhs=ht_sb[:, cc, o:o + n],
                            start=(cc == 0),
                            stop=(cc == CC - 1),
                        )
                    if drain_state[0] % 2 == 0:
                        nc.vector.tensor_copy(yt[:, dblk, :], ps_y[:, 0:n])
                    else:
                        nc.scalar.activation(
                            yt[:, dblk, :], ps_y[:, 0:n], ACT.Copy)
                    drain_state[0] += 1
                    if dblk == DB // 2 - 1 or dblk == DB - 1:
                        h0 = 0 if dblk == DB // 2 - 1 else DB // 2
                        eng = nc.scalar if (last and h0) else nc.sync
                        eng.dma_start(
                            out[:, DB * o + h0 * n:DB * o + (dblk + 1) * n],
                            yt[:, h0:dblk + 1, :])

            # Interleave with one chunk of lag: mm2(c) is emitted after
            # mm1(c+1), so mm1(c)'s drains are long done when mm2(c) hits
            # the PE, and output DMA spreads across the whole kernel
            # instead of piling up at the end.
            emit_mm1(0, *chunks[0])
            for ci in range(1, len(chunks)):
                emit_mm1(ci, *chunks[ci])
                emit_mm2(ci - 1, *chunks[ci - 1])
            emit_mm2(len(chunks) - 1, *chunks[-1], last=True)

    _split_excess_waits(nc, mybir)
    return nc


def _get_nc(cap):
    key = ("nc", cap)
    if key not in _CACHE:
        _CACHE[key] = _build_nc(cap)
    return _CACHE[key]


def kernel(**inputs) -> np.ndarray:
    global LAST_RESULT
    x = np.asarray(inputs["x"], dtype=np.float32)
    Wg = np.asarray(inputs["Wg"], dtype=np.float32)
    W1 = np.asarray(inputs["W1"], dtype=np.float32)
    W2 = np.asarray(inputs["W2"], dtype=np.float32)

    B, S, D = x.shape
    T = B * S
    xf = x.reshape(T, D)

    # ---- routing on host in float64 (logit gaps >> fp32 matmul noise, so
    # this reproduces the reference's fp32 top-2 decisions exactly)
    logits = xf.astype(np.float64) @ Wg.astype(np.float64)
    rows = np.arange(T)
    i1 = logits.argmax(1)
    l1 = logits[rows, i1]
    masked = logits.copy()
    masked[rows, i1] = -np.inf
    i2 = masked.argmax(1)
    l2 = masked[rows, i2]
    p2 = 1.0 / (1.0 + np.exp(l1 - l2))
    p1 = 1.0 - p2

    # ---- dispatch: per-expert token lists + slot of each token in its
    # expert's buffer (for the combine gather)
    slot1 = np.empty(T, np.int64)
    slot2 = np.empty(T, np.int64)
    idx_e = []
    gates_e = []
    for e in range(N_EXP):
        a = np.nonzero(i1 == e)[0]
        b = np.nonzero(i2 == e)[0]
        slot1[a] = np.arange(len(a))
        slot2[b] = len(a) + np.arange(len(b))
        idx_e.append(np.concatenate([a, b]))
        gates_e.append(np.concatenate([p1[a], p2[b]]).astype(np.float32))
    cap = max(max(len(ix) for ix in idx_e), 64)

    chunks = _make_chunks(cap)
    in_maps = []
    for e in range(N_EXP):
        n_e = len(idx_e[e])
        xg = np.zeros((D_MODEL, cap), np.float16)
        xg[:, :n_e] = xf[idx_e[e]].T.astype(np.float16)
        # chunk-major layout: each chunk contiguous per partition as
        # [p][kc][t-in-chunk]
        xgp = np.concatenate(
            [
                xg[:, o:o + n].reshape(KC, P, n)
                .transpose(1, 0, 2).reshape(P, KC * n)
                for o, n in chunks
            ],
            axis=1,
        )
        in_maps.append({
            "xgp": np.ascontiguousarray(xgp),
            "w1p": np.ascontiguousarray(
                W1[e].astype(np.float16)
                .reshape(KC, P, CC, P).transpose(2, 1, 0, 3)
                .reshape(CC, P, KC * P)),
            "w2p": np.ascontiguousarray(
                W2[e].astype(np.float16)
                .reshape(CC, P, D_MODEL).transpose(1, 0, 2)
                .reshape(P, CC * D_MODEL)),
        })

    from concourse.bass_utils import run_bass_kernel_spmd

    _install_ntff_hook_shim()
    nc = _get_nc(cap)
    res = run_bass_kernel_spmd(
        nc, in_maps, core_ids=list(range(N_CORES)), trace=TRACE
    )
    LAST_RESULT = res

    # ---- decode the chunk-major outputs back to [E, D, cap]
    y_all = np.empty((N_EXP, D_MODEL, cap), np.float16)
    for e in range(N_EXP):
        raw = res.results[e]["out"]  # [P, DB*cap] chunk-major
        for o, n in chunks:
            y_all[e, :, o:o + n] = (
                raw[:, DB * o:DB * (o + n)].reshape(P, DB, n)
                .transpose(1, 0, 2).reshape(D_MODEL, n))

    # ---- combine: token t = p1*y[e1] + p2*y[e2] via two fancy-indexed
    # gathers; gate probabilities applied here in fp32.
    c1 = y_all[i1, :, slot1].astype(np.float32)        # [T, D]
    c2 = y_all[i2, :, slot2].astype(np.float32)
    out = p1[:, None].astype(np.float32) * c1 + p2[:, None].astype(np.float32) * c2
    return out.astype(np.float32).reshape(B, S, D)
